# revision 1
# baseline (speedup 1.0000x reference)
"""Trainium2 Bass kernel for nn_LongShortAttention (sparse local+global attention).

Sharding: 8 NeuronCores; core c owns batch c//4, tokens [(c%4)*1024, +1024) with a
128-token left halo.  Per core: Q/KV projections (bf16 weights/activations into
f32 PSUM), windowed local attention, compressed global attention (own 64
segments compressed+LayerNormed, AllGathered across the 4 cores of the batch in
bf16), fused softmax over [global | local] keys, and the output projection.

Key structural choices vs a direct translation of the reference:
- Mean-centering of the local/global LayerNorms is folded into Wkv host-side
  (Wkv' = Wkv @ (I - blockdiag(ones/64))), so both LayerNorms reduce to a pure
  per-token scale rstd = (mean(kv_c^2)+eps)^-1/2.  The segment-softmax logits z
  (which need the *uncentered* kv) come from 16 extra projection columns
  appended to Wkv host-side.
- rstd is applied to the keys in d-major form by a PE row-broadcast matmul
  (block-diagonal selector) followed by one in-place multiply, and to the
  values during the PSUM->SBUF copy of the token-major transpose.  rstd/p/sd
  rows move to token-major columns with PE transposes (no DRAM round trips).
- The compressed-KV branch is kept d-major end to end: compress matmuls emit
  [d, seg], global LN stats come from a ones-row matmul, and the AllGather
  payload is [h, d, s] so the gathered keys DMA straight into the d-major
  gkvT; the seg-major values (gv) are rebuilt with PE transposes.
- All scalar-engine activations stay within one act-table set (exp/ln/copy):
  rstd = exp(-0.5*ln(e2+eps)), 1/Z via vector reciprocal + PE row-broadcast.
- q/kv/attention weights/values and Wo run in bf16; all logits accumulate f32.
"""
import contextlib

import numpy as np
import ml_dtypes

import concourse.bass as bass
import concourse.mybir as mybir
import concourse.tile as tile
from concourse import bacc
from concourse.bass_utils import run_bass_kernel_spmd

A = mybir.AluOpType
AF = mybir.ActivationFunctionType
F32 = mybir.dt.float32
F32R = mybir.dt.float32r
BF16 = mybir.dt.bfloat16

B, N, DIM, H, D = 2, 4096, 1024, 16, 64
W, S, R = 128, 16, 1
EPS = 1e-5
SCALE = D ** -0.5
NC = 8
TOK = 1024
HALO = 128
TOKH = 1152
NT = 9                      # token tiles incl halo (tt=0 is halo)
NSEG = TOKH // S            # 72 segments incl halo
P = 128
BF = ml_dtypes.bfloat16


def build_program(nonzero_bq=False, nonzero_bkv=False, nonzero_bo=False):
    nc = bacc.Bacc(None, target_bir_lowering=False, debug=False)

    xt = nc.declare_dram_parameter("xt", [DIM, TOKH], BF16, isOutput=False)
    wq = nc.declare_dram_parameter("wq", [DIM, DIM], BF16, isOutput=False)
    wkva = nc.declare_dram_parameter("wkva", [DIM, DIM + 16], BF16,
                                     isOutput=False)
    wo = nc.declare_dram_parameter("wo", [DIM, DIM], BF16, isOutput=False)
    identf_d = nc.declare_dram_parameter("identf", [P, P], F32R, isOutput=False)
    identb_d = nc.declare_dram_parameter("identb", [P, P], BF16, isOutput=False)
    seg16_d = nc.declare_dram_parameter("seg16b", [P, 16, 8], BF16,
                                        isOutput=False)
    tri_d = nc.declare_dram_parameter("trib", [P, P], BF16, isOutput=False)
    halo_d = nc.declare_dram_parameter("halob", [P, P], BF16, isOutput=False)
    gmask_d = nc.declare_dram_parameter("gmaskb", [P, 2, 2, 512], BF16,
                                        isOutput=False)
    ssel_d = nc.declare_dram_parameter("stats_sel", [P, 30], F32R,
                                       isOutput=False)
    selS_d = nc.declare_dram_parameter("selS", [16, DIM], F32R, isOutput=False)
    onesc_d = nc.declare_dram_parameter("onesc", [64, 1], F32R, isOutput=False)
    if nonzero_bq:
        bq_d = nc.declare_dram_parameter("bqs", [P, 8], F32, isOutput=False)
    if nonzero_bkv:
        bkv_d = nc.declare_dram_parameter("bkvs", [P, 8], F32, isOutput=False)
    if nonzero_bo:
        bo_d = nc.declare_dram_parameter("bob", [1, DIM], BF16, isOutput=False)
    out_d = nc.declare_dram_parameter("out", [8, P, DIM], BF16,
                                  isOutput=True)

    with tile.TileContext(nc) as tc:
        stack = contextlib.ExitStack()
        with stack:
            dram = stack.enter_context(tc.tile_pool(name="dram", bufs=1,
                                                    space="DRAM"))
            consts = stack.enter_context(tc.tile_pool(name="consts", bufs=1))

            if nonzero_bq:
                bqs = consts.tile([P, 8], F32)
                nc.sync.dma_start(out=bqs[:], in_=bq_d[:])
            if nonzero_bkv:
                bkvs = consts.tile([P, 8], F32)
                nc.sync.dma_start(out=bkvs[:], in_=bkv_d[:])
            if nonzero_bo:
                bob = consts.tile([1, DIM], BF16)
                nc.sync.dma_start(out=bob[:], in_=bo_d[:])
                ones1b = consts.tile([1, P], BF16)
                nc.vector.memset(ones1b[:], 1.0)

            cc_in = dram.tile([16, 64, 64], BF16)      # [h, d, seg]
            cc_out = dram.tile([4, 16, 64, 64], BF16)  # [member, h, d, seg]

            pool_qT = stack.enter_context(tc.tile_pool(name="p_qT", bufs=1))
            pool_kvT = stack.enter_context(
                tc.tile_pool(name="p_kvT", bufs=1, side="right"))
            pool_rows = stack.enter_context(
                tc.tile_pool(name="p_rows", bufs=1, side="right"))
            pool_attn = stack.enter_context(tc.tile_pool(name="p_attn",
                                                         bufs=1))

            qT = pool_qT.tile([P, 8, TOK], BF16)      # [d-in-m, m, tok]
            kvT = pool_kvT.tile([P, 8, TOKH], BF16)   # [d-in-m, m, tok+halo]
            v_ln = pool_attn.tile([P, NT, 16, 65], BF16)   # token-major LN'd kv
            gv = pool_attn.tile([P, 2, 16, 65], BF16)      # [seg, bb, h, d+1]
            gkvT = pool_attn.tile([P, 2, 8, P], BF16)      # [d(2h), bb, m, seg]
            zq = pool_rows.tile([16, TOKH], F32)           # z logits rows
            rpstack = pool_rows.tile([80, TOKH], F32R)      # rstd|_|p|_|sd rows
            colsb = pool_rows.tile([P, NT, 80], F32)       # token-major columns
            psd = pool_rows.tile([P, NT, 16], F32)         # p * sd columns
            gall = pool_rows.tile([64, 16, 64], F32)       # own gkv, d-major

            nc.vector.memset(v_ln[:, :, :, 64:65], 1.0)
            nc.vector.memset(gv[:, :, :, 64:65], 1.0)

            # ---------------- Phase B: KV/Z projections ----------------
            bload = contextlib.ExitStack()
            xw_pool = bload.enter_context(tc.tile_pool(name="xw", bufs=8))
            wld_pool = bload.enter_context(tc.tile_pool(name="wld", bufs=8))

            xt_k = []
            for k in range(8):
                xk = xw_pool.tile([P, TOKH], BF16, tag="xk", name="xk")
                nc.sync.dma_start(out=xk[:], in_=xt[k * P:(k + 1) * P, :])
                xt_k.append(xk)
            wkv_k = []
            for k in range(8):
                wk2 = wld_pool.tile([P, DIM + 16], BF16, tag="wkv", name="wkv")
                nc.sync.dma_start(out=wk2[:], in_=wkva[k * P:(k + 1) * P, :])
                wkv_k.append(wk2)
            identf = consts.tile([P, P], F32R)
            nc.sync.dma_start(out=identf[:], in_=identf_d[:])
            identb = consts.tile([P, P], BF16)
            nc.sync.dma_start(out=identb[:], in_=identb_d[:])
            seg16b = consts.tile([P, 16, 8], BF16)
            nc.sync.dma_start(out=seg16b[:], in_=seg16_d[:])
            trib = consts.tile([P, P], BF16)
            nc.sync.dma_start(out=trib[:], in_=tri_d[:])
            halob = consts.tile([P, P], BF16)
            nc.sync.dma_start(out=halob[:], in_=halo_d[:])
            gmaskb = consts.tile([P, 2, 2, 512], BF16)
            nc.sync.dma_start(out=gmaskb[:], in_=gmask_d[:])
            ssel = consts.tile([P, 30], F32R)
            nc.sync.dma_start(out=ssel[:], in_=ssel_d[:])
            selS = consts.tile([16, DIM], F32R)
            nc.sync.dma_start(out=selS[:], in_=selS_d[:])
            eps_t = consts.tile([P, 1], F32)
            nc.vector.memset(eps_t[:], EPS)
            onesT64 = consts.tile([64, 1], F32R)
            nc.sync.dma_start(out=onesT64[:], in_=onesc_d[:])
            wq_k = []
            for k in range(8):
                wk3 = wld_pool.tile([P, DIM], BF16, tag="wq", name="wq")
                nc.sync.dma_start(out=wk3[:], in_=wq[k * P:(k + 1) * P, :])
                wq_k.append(wk3)

            with tc.tile_pool(name="pproj", bufs=3, space="PSUM") as pproj, \
                 tc.tile_pool(name="pzpr", bufs=1, space="PSUM") as pzpr:
                for m in range(8):
                    for nt in range(3):
                        ps = pproj.tile([P, 384], F32, tag="proj", name="ps")
                        for k in range(8):
                            nc.tensor.matmul(
                                ps[:], wkv_k[k][:, m * P:(m + 1) * P],
                                xt_k[k][:, nt * 384:nt * 384 + 384],
                                start=(k == 0), stop=(k == 7))
                        dst = kvT[:, m, nt * 384:(nt + 1) * 384]
                        if nonzero_bkv:
                            nc.scalar.activation(dst, ps[:], AF.Identity,
                                                 bias=bkvs[:, m:m + 1])
                        else:
                            nc.scalar.activation(dst, ps[:], AF.Copy)
                for nt in range(3):
                    psz = pzpr.tile([16, 384], F32, tag="zproj", name="psz")
                    for k in range(8):
                        nc.tensor.matmul(
                            psz[:], wkv_k[k][:, DIM:DIM + 16],
                            xt_k[k][:, nt * 384:nt * 384 + 384],
                            start=(k == 0), stop=(k == 7))
                    nc.scalar.activation(zq[:, nt * 384:(nt + 1) * 384],
                                         psz[:], AF.Copy)

            # ------- Phase C: stats (e2), z-softmax, token-major columns ------
            with tc.tile_pool(name="sq", bufs=2) as sq_pool, \
                 tc.tile_pool(name="pstat", bufs=1, space="PSUM") as pstat, \
                 tc.tile_pool(name="pcolT", bufs=1, space="PSUM") as pcolT:
                pse = [pstat.tile([16, 384], F32, tag=f"pse{nt}",
                                  name=f"pse{nt}") for nt in range(3)]
                for m in range(8):
                    sqt = sq_pool.tile([P, TOKH], F32R, tag="sqt", name="sqt")
                    nc.gpsimd.tensor_tensor(out=sqt[:], in0=kvT[:, m, :],
                                            in1=kvT[:, m, :], op=A.mult)
                    for nt in range(3):
                        nc.tensor.matmul(
                            pse[nt][:], ssel[:, 14 - 2 * m:30 - 2 * m],
                            sqt[:, nt * 384:nt * 384 + 384],
                            start=(m == 0), stop=(m == 7))
                lnt = pool_rows.tile([16, TOKH], F32)
                for nt in range(3):
                    nc.scalar.activation(lnt[:, nt * 384:(nt + 1) * 384],
                                         pse[nt][:], AF.Ln, bias=eps_t[0:16])
                nc.scalar.activation(rpstack[0:16, :], lnt[:], AF.Exp,
                                     scale=-0.5)
                nc.scalar.activation(rpstack[64:80, :], lnt[:], AF.Exp,
                                     scale=0.5)
                # z segment softmax
                esz = pool_rows.tile([16, TOKH], F32)
                nc.scalar.activation(esz[:], zq[:], AF.Exp)
                szr = pool_rows.tile([16, NSEG], F32)
                nc.vector.reduce_sum(
                    szr[:], esz[:].rearrange("p (g s) -> p g s", s=S),
                    axis=mybir.AxisListType.X)
                lsz = pool_rows.tile([16, NSEG], F32)
                nc.scalar.activation(lsz[:], szr[:], AF.Ln)
                rsz = pool_rows.tile([16, NSEG], F32)
                nc.scalar.activation(rsz[:], lsz[:], AF.Exp, scale=-1.0)
                with nc.allow_low_precision(reason="f32r rows"):
                    nc.vector.tensor_tensor(
                        out=rpstack[32:48, :].rearrange("p (g s) -> p g s",
                                                        s=S),
                        in0=esz[:].rearrange("p (g s) -> p g s", s=S),
                        in1=rsz[:].unsqueeze(2).broadcast_to([16, NSEG, S]),
                        op=A.mult)
                # rows -> token-major columns via PE transposes
                colpsA = pcolT.tile([P, 4, 80], F32R, name="colpsA")
                colpsB = pcolT.tile([P, 5, 80], F32R, name="colpsB")
                for tt in range(NT):
                    dstc = colpsA[:, tt, :] if tt < 4 else colpsB[:, tt - 4, :]
                    nc.tensor.transpose(
                        dstc, rpstack[:, tt * P:(tt + 1) * P],
                        identf[0:80, 0:80])
                nc.vector.tensor_copy(out=colsb[:, 0:4, :].bitcast(F32R),
                                      in_=colpsA[:])
                nc.vector.tensor_copy(out=colsb[:, 4:9, :].bitcast(F32R),
                                      in_=colpsB[:])
                nc.vector.tensor_tensor(out=psd[:], in0=colsb[:, :, 32:48],
                                        in1=colsb[:, :, 64:80], op=A.mult)

            # ------- Phase D: token-major transpose, v_ln, compress -------
            with tc.tile_pool(name="ptok", bufs=4, space="PSUM") as ptokp, \
                 tc.tile_pool(name="pg", bufs=2, space="PSUM") as pgp, \
                 tc.tile_pool(name="wsg", bufs=2) as wsgp:
                for tt in list(range(1, NT)) + [0]:
                    if tt >= 1:
                        wsg = wsgp.tile([P, 16, 8], BF16, tag="wsg", name="wsg")
                        with nc.allow_low_precision(reason="bf16 weights"):
                            nc.vector.tensor_tensor(
                                out=wsg[:], in0=seg16b[:],
                                in1=psd[:, tt, :].unsqueeze(2).broadcast_to(
                                    [P, 16, 8]),
                                op=A.mult)
                        pgd = pgp.tile([64, 16, 8], F32, tag="pgd", name="pgd")
                    for m in range(8):
                        ptok = ptokp.tile([P, P], BF16, tag="ptok", name="ptok")
                        nc.tensor.transpose(
                            ptok[:], kvT[:, m, tt * P:(tt + 1) * P], identb[:])
                        with nc.allow_low_precision(reason="bf16 values"):
                            nc.vector.tensor_tensor(
                                out=v_ln[:, tt, 2 * m:2 * m + 2, 0:64],
                                in0=ptok[:].rearrange("p (a d) -> p a d", a=2),
                                in1=colsb[:, tt, 2 * m:2 * m + 2].unsqueeze(2)
                                .broadcast_to([P, 2, 64]),
                                op=A.mult)
                    if tt >= 1:
                        for m in range(8):
                            for par in range(2):
                                h = 2 * m + par
                                nc.tensor.matmul(
                                    pgd[:, h, :], v_ln[:, tt, h, 0:64],
                                    wsg[:, h, :], start=True, stop=True)
                        nc.vector.tensor_copy(
                            out=gall[:, :, 8 * (tt - 1):8 * tt], in_=pgd[:])

            # global LN of compressed kv + AllGather (launch early)
            with tc.tile_pool(name="glnp", bufs=1) as gln_pool, \
                 tc.tile_pool(name="pgs", bufs=2, space="PSUM") as pgsp:
                gsq = gln_pool.tile([64, 16, 64], F32R, name="gsq")
                with nc.allow_low_precision(reason="f32r stats"):
                    nc.vector.tensor_tensor(out=gsq[:], in0=gall[:],
                                            in1=gall[:], op=A.mult)
                lnr = gln_pool.tile([1, 1024], F32, name="lnr")
                rpr = gln_pool.tile([1, 1024], F32R, name="rpr")
                gsqf = gsq[:].rearrange("p a b -> p (a b)")
                for c2 in range(2):
                    pe2 = pgsp.tile([1, 512], F32, tag="pe2", name="pe2")
                    nc.tensor.matmul(
                        pe2[:], onesT64[:],
                        gsqf[:, c2 * 512:(c2 + 1) * 512],
                        start=True, stop=True)
                    nc.scalar.activation(lnr[:, c2 * 512:(c2 + 1) * 512],
                                         pe2[:], AF.Ln, bias=eps_t[0:1],
                                         scale=1.0 / 64)
                nc.scalar.activation(rpr[:], lnr[:], AF.Exp, scale=-0.5)
                gstage = gln_pool.tile([64, 16, 64], BF16, name="gstage")
                for c2 in range(2):
                    rbg = pgsp.tile([64, 512], F32, tag="rbg", name="rbg")
                    nc.tensor.matmul(
                        rbg[:], selS[0:1, 0:64],
                        rpr[:, c2 * 512:(c2 + 1) * 512],
                        start=True, stop=True)
                    with nc.allow_low_precision(reason="bf16 collective"):
                        nc.vector.tensor_tensor(
                            out=gstage[:, c2 * 8:(c2 + 1) * 8, :],
                            in0=gall[:, c2 * 8:(c2 + 1) * 8, :],
                            in1=rbg[:].rearrange("p (a b) -> p a b", a=8),
                            op=A.mult)
                # payload [h, d, s] <- gstage [d, h, s]
                nc.sync.dma_start(out=cc_in[:].transpose([1, 0, 2]),
                                  in_=gstage[:])
                nc.gpsimd.collective_compute(
                    "AllGather", A.bypass,
                    replica_groups=[[0, 1, 2, 3], [4, 5, 6, 7]],
                    ins=[cc_in.opt()], outs=[cc_out.opt()])

            # ---------------- Q projection (overlaps collective) ----------
            with tc.tile_pool(name="pq", bufs=3, space="PSUM") as pqp:
                for m in range(8):
                    for nt2 in range(2):
                        psq = pqp.tile([P, 512], F32, tag="psq", name="psq")
                        for k in range(8):
                            nc.tensor.matmul(
                                psq[:], wq_k[k][:, m * P:(m + 1) * P],
                                xt_k[k][:, HALO + nt2 * 512:
                                        HALO + nt2 * 512 + 512],
                                start=(k == 0), stop=(k == 7))
                        dst = qT[:, m, nt2 * 512:(nt2 + 1) * 512]
                        if nonzero_bq:
                            nc.scalar.activation(dst, psq[:], AF.Identity,
                                                 bias=bqs[:, m:m + 1])
                        else:
                            nc.scalar.activation(dst, psq[:], AF.Copy)

            bload.close()   # free x / wkv / wq SBUF

            # in-place LN scale of kv keys in d-major (row-broadcast via PE)
            with tc.tile_pool(name="prb", bufs=2, space="PSUM") as prbp:
                for m in range(8):
                    for nt in range(3):
                        rbp = prbp.tile([P, 384], F32, tag="rb", name="rbp")
                        nc.tensor.matmul(
                            rbp[:], selS[:, m * P:(m + 1) * P],
                            rpstack[0:16, nt * 384:nt * 384 + 384],
                            start=True, stop=True)
                        with nc.allow_low_precision(reason="bf16 store"):
                            nc.vector.tensor_tensor(
                                out=kvT[:, m, nt * 384:(nt + 1) * 384],
                                in0=kvT[:, m, nt * 384:(nt + 1) * 384],
                                in1=rbp[:], op=A.mult)

            # unpack AllGather: keys straight into d-major gkvT
            for bb in range(2):
                for cg in range(2):
                    for m in range(8):
                        for par in range(2):
                            nc.sync.dma_start(
                                out=gkvT[64 * par:64 * par + 64, bb, m,
                                         64 * cg:64 * cg + 64],
                                in_=cc_out[2 * bb + cg][2 * m + par])
            # ---------------- Phase E: attention per head-pair ----------------
            pool_out = stack.enter_context(tc.tile_pool(name="p_out", bufs=1))
            attnT = pool_out.tile([P, 8, TOK], BF16)
            wof_pool = stack.enter_context(tc.tile_pool(name="wof", bufs=8))
            wo_k = []
            for k in range(8):
                wk4 = wof_pool.tile([P, DIM], BF16, tag="wo", name="wo")
                nc.sync.dma_start(out=wk4[:], in_=wo[k * P:(k + 1) * P, :])
                wo_k.append(wk4)
            gmr = gmaskb[:].rearrange("p a b c -> p a (b c)")
            with tc.tile_pool(name="expl", bufs=3) as explp, \
                 tc.tile_pool(name="expg", bufs=3) as expgp, \
                 tc.tile_pool(name="plsim", bufs=2, space="PSUM") as plsim, \
                 tc.tile_pool(name="pgsim", bufs=1, space="PSUM") as pgsim, \
                 tc.tile_pool(name="pav", bufs=4, space="PSUM") as pav, \
                 tc.tile_pool(name="evs", bufs=2) as evs:
                def do_local(m):
                    expL = explp.tile([P, 2, NT, 256], BF16, tag="expL",
                                      name="expL")
                    for par in range(2):
                        prow = slice(par * 64, par * 64 + 64)
                        for up in range(4):
                            # pair key blocks (2up, 2up+1) in one PSUM bank
                            psl = plsim.tile([P, 2, 256], F32, tag="psl",
                                             name="psl")
                            for j in range(2):
                                u = 2 * up + j
                                qs = 0 if u == 0 else (u - 1) * P
                                nc.tensor.matmul(
                                    psl[:, j, :],
                                    kvT[prow, m, u * P:(u + 1) * P],
                                    qT[prow, m, qs:qs + 256],
                                    start=True, stop=True)
                            nc.scalar.activation(
                                expL[:, par, 2 * up:2 * up + 2, :], psl[:],
                                AF.Exp)
                        psl8 = plsim.tile([P, 2, 256], F32, tag="psl",
                                          name="psl8")
                        nc.tensor.matmul(
                            psl8[:, 0, :], kvT[prow, m, 8 * P:9 * P],
                            qT[prow, m, 768:1024], start=True, stop=True)
                        nc.scalar.activation(expL[:, par, 8, 128:256],
                                             psl8[:, 0, 128:256], AF.Exp)
                    for u in range(NT):
                        msk = halob if u == 0 else trib
                        cs = 128 if u == 8 else 0
                        with nc.allow_low_precision(reason="bf16 weights"):
                            nc.vector.tensor_tensor(
                                out=expL[:, :, u, cs:cs + 128],
                                in0=expL[:, :, u, cs:cs + 128],
                                in1=msk[:].unsqueeze(1).broadcast_to(
                                    [P, 2, 128]),
                                op=A.mult)
                    return expL

                # hoist m=0,1 local sims to overlap the AllGather latency
                hoisted = {m: do_local(m) for m in range(2)}

                # seg-major values gv via PE transposes of gkvT
                with tc.tile_pool(name="pgt", bufs=1, space="PSUM") as pgt:
                    for bb in range(2):
                        for par in range(2):
                            for mg in range(2):
                                pst = pgt.tile([P, 4, 64], BF16, tag="pgt",
                                               name="pst")
                                for j in range(4):
                                    mj = 4 * mg + j
                                    nc.tensor.transpose(
                                        pst[:, j, :],
                                        gkvT[64 * par:64 * par + 64, bb,
                                             mj, :],
                                        identb[64 * par:64 * par + 64,
                                               64 * par:64 * par + 64])
                                nc.vector.tensor_copy(
                                    out=gv[:, bb, 8 * mg + par:
                                           8 * mg + par + 7:2, 0:64],
                                    in_=pst[:])

                def do_global(m):
                    expG = expgp.tile([P, 2, 2, TOK], BF16, tag="expG",
                                      name="expG")
                    for bb in range(2):
                        for Qh in range(2):
                            for par in range(2):
                                prow = slice(par * 64, par * 64 + 64)
                                psg = pgsim.tile([P, 512], F32,
                                                 tag="psg", name="psg")
                                nc.tensor.matmul(
                                    psg[:], gkvT[prow, bb, m, :],
                                    qT[prow, m, Qh * 512:Qh * 512 + 512],
                                    start=True, stop=True)
                                nc.scalar.activation(
                                    expG[:, par, bb,
                                         Qh * 512:Qh * 512 + 512],
                                    psg[:], AF.Exp)
                    for par in range(2):
                        with nc.allow_low_precision(reason="bf16 weights"):
                            nc.vector.tensor_tensor(
                                out=expG[:, par], in0=expG[:, par], in1=gmr,
                                op=A.mult)
                    return expG

                gdone = {0: do_global(0)}
                for m in range(8):
                    expL = hoisted.pop(m)
                    expG = gdone.pop(m)
                    # pipeline: next head's global sims and next+1 head's
                    # local sims fill the PE while this head's exp/mask
                    # chain drains on scalar/vector
                    if m + 1 < 8:
                        gdone[m + 1] = do_global(m + 1)
                    if m + 2 < 8:
                        hoisted[m + 2] = do_local(m + 2)
                    # AV + Z, normalize
                    for par in range(2):
                        h = 2 * m + par
                        prow = slice(par * 64, par * 64 + 64)
                        for Q in range(2):
                            avp = pav.tile([65, 512], F32, tag="avp",
                                           name="avp")
                            nc.tensor.matmul(avp[:], gv[:, 0, h, :],
                                             expG[:, par, 0,
                                                  Q * 512:(Q + 1) * 512],
                                             start=True, stop=False)
                            nc.tensor.matmul(avp[:], gv[:, 1, h, :],
                                             expG[:, par, 1,
                                                  Q * 512:(Q + 1) * 512],
                                             start=False, stop=False)
                            if Q == 0:
                                mm_list = [(0, 0, 128, 0), (1, 0, 256, 0),
                                           (2, 0, 256, 128), (3, 0, 256, 256),
                                           (4, 0, 128, 384)]
                            else:
                                mm_list = [(4, 128, 128, 0), (5, 0, 256, 0),
                                           (6, 0, 256, 128), (7, 0, 256, 256),
                                           (8, 128, 128, 384)]
                            for idx, (u, cs, cn, po) in enumerate(mm_list):
                                nc.tensor.matmul(
                                    avp[:, po:po + cn], v_ln[:, u, h, :],
                                    expL[:, par, u, cs:cs + cn],
                                    start=False,
                                    stop=(idx == len(mm_list) - 1))
                            zsb = evs.tile([1, 512], F32, tag="zsb",
                                           name="zsb")
                            nc.vector.tensor_copy(out=zsb[:],
                                                  in_=avp[64:65, :])
                            rcp = evs.tile([1, 512], F32, tag="rcp",
                                           name="rcp")
                            nc.vector.reciprocal_approx_fast(
                                out=rcp[:], in_=zsb[:])
                            rzb = evs.tile([64, 512], F32, tag="rzb",
                                           name="rzb")
                            nc.gpsimd.partition_broadcast(rzb[:], rcp[:])
                            with nc.allow_low_precision(reason="bf16 out"):
                                nc.vector.tensor_tensor(
                                    out=attnT[prow, m, Q * 512:(Q + 1) * 512],
                                    in0=avp[0:64, :], in1=rzb[:], op=A.mult)

            # ---------------- Phase F: output projection ----------------
            with tc.tile_pool(name="pf", bufs=3, space="PSUM") as pf, \
                 tc.tile_pool(name="outp", bufs=2) as outp:
                for tt in range(8):
                    ot = outp.tile([P, DIM], BF16, tag="ot", name="ot")
                    for nh in range(2):
                        psf = pf.tile([P, 512], F32, tag="psf", name="psf")
                        for m in range(8):
                            nc.tensor.matmul(
                                psf[:], attnT[:, m, tt * P:(tt + 1) * P],
                                wo_k[m][:, nh * 512:(nh + 1) * 512],
                                start=(m == 0),
                                stop=(m == 7 and not nonzero_bo))
                        if nonzero_bo:
                            nc.tensor.matmul(
                                psf[:], ones1b[:],
                                bob[:, nh * 512:(nh + 1) * 512],
                                start=False, stop=True)
                        nc.scalar.activation(ot[:, nh * 512:(nh + 1) * 512],
                                             psf[:], AF.Copy)
                    nc.sync.dma_start(out=out_d[tt], in_=ot[:])

    nc.compile()
    return nc


_PROG_CACHE = {}


def _get_program(key):
    if key not in _PROG_CACHE:
        _PROG_CACHE[key] = build_program(*key)
    return _PROG_CACHE[key]


def _host_constants(Wq, Wkv, Wp):
    # centering matrix folded into Wkv; z-logit columns appended
    Cm = np.eye(DIM, dtype=np.float64)
    for h in range(H):
        Cm[h * D:(h + 1) * D, h * D:(h + 1) * D] -= 1.0 / D
    Wkv_c = (Wkv.astype(np.float64) @ Cm).astype(np.float32)
    Wz = np.stack([Wkv[:, h * D:(h + 1) * D].astype(np.float64) @
                   Wp[:, 0].astype(np.float64) for h in range(H)],
                  axis=1).astype(np.float32)
    wkva = np.concatenate([Wkv_c, Wz], axis=1).astype(BF)
    wq_eff = (Wq * SCALE).astype(BF)
    identf = np.eye(P, dtype=np.float32)
    identb = np.eye(P).astype(BF)
    seg16 = np.zeros((P, 8), np.float32)
    for g in range(8):
        seg16[g * 16:(g + 1) * 16, g] = 1.0
    seg16 = np.ascontiguousarray(
        np.broadcast_to(seg16[:, None, :], (P, 16, 8)))
    jk, ii = np.meshgrid(np.arange(P), np.arange(P), indexing="ij")
    tri = (jk <= ii).astype(BF)
    ssel = np.zeros((P, 30), np.float32)
    ssel[0:64, 14] = 1.0 / 64
    ssel[64:128, 15] = 1.0 / 64
    selS = np.zeros((16, DIM), np.float32)
    for h in range(16):
        selS[h, 64 * h:64 * h + 64] = 1.0
    return wkva, wq_eff, identf, identb, seg16.astype(BF), tri, ssel, selS


def kernel(x, Wq, bq, Wkv, bkv, Wp, bp, ln_l_g, ln_l_b, ln_g_g, ln_g_b, Wo, bo):
    # NOTE: bp shifts all segment logits equally (R=1), so the segment softmax
    # is invariant to it; it is deliberately unused.
    x = np.ascontiguousarray(x, np.float32)
    Wq = np.asarray(Wq, np.float32); Wkv = np.asarray(Wkv, np.float32)
    Wo = np.asarray(Wo, np.float32); Wp = np.asarray(Wp, np.float32)
    bq = np.asarray(bq, np.float32); bkv = np.asarray(bkv, np.float32)
    bo = np.asarray(bo, np.float32)
    assert Wp.shape[1] == 1, "only rank-1 dynamic projection supported"
    assert (np.all(np.asarray(ln_l_g) == 1.0)
            and np.all(np.asarray(ln_l_b) == 0.0)
            and np.all(np.asarray(ln_g_g) == 1.0)
            and np.all(np.asarray(ln_g_b) == 0.0)), \
        "nontrivial LayerNorm affine not supported"

    nonzero_bq = bool(np.any(bq != 0.0))
    nonzero_bkv = bool(np.any(bkv != 0.0))
    nonzero_bo = bool(np.any(bo != 0.0))
    key = (nonzero_bq, nonzero_bkv, nonzero_bo)
    nc = _get_program(key)

    wkva, wq_eff, identf, identb, seg16, tri, ssel, selS = \
        _host_constants(Wq, Wkv, Wp)
    wo_b = Wo.astype(BF)

    in_maps = []
    for c in range(NC):
        bc, ci = c // 4, c % 4
        tc0 = ci * TOK
        xb = x[bc]
        xtc = np.zeros((DIM, TOKH), np.float32)
        lo = tc0 - HALO
        src_lo = max(lo, 0)
        xtc[:, src_lo - lo:] = xb[src_lo:tc0 + TOK].T
        halom = ((np.ones if ci > 0 else np.zeros)((P, P))).astype(BF)
        qi = tc0 + np.arange(1024).reshape(2, 512)
        seg = np.arange(256).reshape(2, 128)
        gm = (qi[None, :, None, :] >= (16 * seg[:, None, :, None] + 15))
        gmask = np.ascontiguousarray(gm.transpose(2, 0, 1, 3)).astype(BF)
        im = dict(xt=xtc.astype(BF), wq=wq_eff, wkva=wkva, wo=wo_b,
                  identf=identf, identb=identb, seg16b=seg16, trib=tri,
                  halob=halom, gmaskb=gmask, stats_sel=ssel, selS=selS,
                  onesc=np.ones((64, 1), np.float32))
        if nonzero_bq:
            im["bqs"] = np.ascontiguousarray((bq * SCALE).reshape(8, P).T)
        if nonzero_bkv:
            Cmat = np.eye(DIM, dtype=np.float64)
            for h in range(H):
                Cmat[h * D:(h + 1) * D, h * D:(h + 1) * D] -= 1.0 / D
            bkv_c = (bkv.astype(np.float64) @ Cmat).astype(np.float32)
            im["bkvs"] = np.ascontiguousarray(bkv_c.reshape(8, P).T)
        if nonzero_bo:
            im["bob"] = bo.reshape(1, DIM).astype(BF)
        in_maps.append(im)

    res = run_bass_kernel_spmd(nc, in_maps, list(range(NC)))
    out = np.empty((B, N, DIM), np.float32)
    for c in range(NC):
        bc, ci = c // 4, c % 4
        out[bc, ci * TOK:(ci + 1) * TOK] = np.asarray(
            res.results[c]["out"], dtype=np.float32).reshape(TOK, DIM)
    return out

